# revision 1
# baseline (speedup 1.0000x reference)
"""Causal single-head attention [Sq,B,D]=[2048,4,512] fp32 on 8 TRN2 NeuronCores.

Sharding: core = 2*b + p  (b = batch 0..3, p = query-row parity).
Core (b, p) computes output rows i = 2j + p (j = 0..1023) of batch b.

Key trick for SPMD (one program, 8 cores): queries are strided by 2, and
K/V are host-shifted by s = 1-p rows. Then the causal condition
  k <= i  ==  k' <= 2*j + 1   (k' = shifted key index)
is identical on every core, so the on-device causal mask is a compile-time
affine_select and block extents are core-invariant.

Math per core: S^T[k',j] = K'^T Q^T / sqrt(D) via PE (contract d);
P^T = exp(S^T) (no max subtraction: scores ~ N(0,1), bounded);
causal zeroing via affine_select; O = P V' and r = P @ ones' accumulated
in PSUM over k' chunks; O /= r. Key-mask + shift padding are folded into
V' rows and ones' (zeroed) on the host, so masked keys contribute 0 to
both numerator and denominator. Matmuls run in float32r (full-rate fp32
storage with TF32-like internal rounding).
"""
import math
import os
import subprocess
from contextlib import ExitStack

import numpy as np

import concourse.bass as bass
import concourse.tile as tile
import concourse.mybir as mybir
from concourse import bacc
from concourse.bass_utils import run_bass_kernel_spmd

SQ, SK, B, D = 2048, 2048, 4, 512
N_CORES = 8
QL = SQ // 2          # local q rows per core
QB = 256              # local q-block size
NBLK = QL // QB       # 4 blocks
NKC = SK // 128       # 16 key chunks
EXT = [4 * (m + 1) for m in range(NBLK)]   # k'-chunk extent per block
BAND = 4              # diagonal band width in chunks
SCALE = 1.0 / math.sqrt(D)

_cache = {}


def _build(stage=4, num_devices=N_CORES, mmdt="float32r"):
    f32 = mybir.dt.float32
    f32r = {"float32r": mybir.dt.float32r,
            "float16": mybir.dt.float16}[mmdt]   # matmul operand dtype
    accdt = mybir.dt.float32r                    # r-accumulator dtype
    Exp = mybir.ActivationFunctionType.Exp

    nc = bacc.Bacc("TRN2", target_bir_lowering=False, debug=False,
                   num_devices=num_devices)
    qt_d = nc.dram_tensor("qt", [D, QL], f32r, kind="ExternalInput").ap()
    kt_d = nc.dram_tensor("kt", [D, SK], f32r, kind="ExternalInput").ap()
    v_d = nc.dram_tensor("v", [SK, D], f32r, kind="ExternalInput").ap()
    bias_d = nc.dram_tensor("bias2d", [128, NKC], f32, kind="ExternalInput").ap()
    onec_d = nc.dram_tensor("onecol", [128, 1], accdt, kind="ExternalInput").ap()
    out_d = nc.dram_tensor("out", [QL, D], f32, kind="ExternalOutput").ap()

    with tile.TileContext(nc) as tc, ExitStack() as ctx:
        const = ctx.enter_context(tc.tile_pool(name="const", bufs=1))
        pin = ctx.enter_context(tc.tile_pool(name="pin", bufs=1))
        ppt = ctx.enter_context(tc.tile_pool(name="ppt", bufs=3))
        pst = ctx.enter_context(tc.tile_pool(name="pst", bufs=4, space="PSUM"))
        pacc = ctx.enter_context(tc.tile_pool(name="pacc", bufs=1, space="PSUM"))
        pfin = ctx.enter_context(tc.tile_pool(name="pfin", bufs=2))

        ident = const.tile([1, 1], f32)
        nc.vector.memset(ident[:], 1.0)
        bias_sb = const.tile([128, NKC], f32)
        nc.sync.dma_start(bias_sb[:], bias_d[:])
        onec_sb = const.tile([128, 1], accdt)
        nc.sync.dma_start(onec_sb[:], onec_d[:])

        # First-needed group (kt g0, qt h0) stays per-dc for sharp deps;
        # later groups merge all 4 dc into one DMA via 3-level APs.
        kt00_sb = pin.tile([128, 128], f32r, tag="kt00", name="kt00")
        ktA0_sb = [pin.tile([128, 512], f32r, tag=f"ktA0_{dc}", name=f"ktA0_{dc}")
                   for dc in range(4)]
        ktA1_sb = pin.tile([128, 4, 512], f32r, tag="ktA1", name="ktA1")
        ktB_sb = pin.tile([128, 4, 1024], f32r, tag="ktB", name="ktB")
        qt0_sb = [pin.tile([128, 512], f32r, tag=f"qt0_{dc}", name=f"qt0_{dc}")
                  for dc in range(4)]
        qt1_sb = pin.tile([128, 4, 512], f32r, tag="qt1", name="qt1")
        vq_sb = [pin.tile([128, 4, 512], f32r, tag=f"vq{g}", name=f"vq{g}")
                 for g in range(4)]

        def kt_slice(dc, c):
            if dc == 0 and c == 0:
                return kt00_sb[:]
            if c < 4:
                return ktA0_sb[dc][:, 128 * c:128 * (c + 1)]
            if c < 8:
                return ktA1_sb[:, dc, 128 * (c - 4):128 * (c - 3)]
            return ktB_sb[:, dc, 128 * (c - 8):128 * (c - 7)]

        def qt_slice(dc, m, width=QB):
            if m < 2:
                return qt0_sb[dc][:, QB * m:QB * m + width]
            return qt1_sb[:, dc, QB * (m - 2):QB * (m - 2) + width]

        def load_vq(g):
            nc.sync.dma_start(
                vq_sb[g][:],
                v_d[512 * g:512 * (g + 1), :].rearrange("(c p) d -> p c d", p=128))

        # All input loads issue from sync (no compute there) in consumption
        # order; engine streams are strict FIFO, so loads must never sit in
        # front of compute ops on scalar/gpsimd.
        nc.sync.dma_start(kt00_sb[:], kt_d[0:128, 0:128])
        nc.sync.dma_start(qt0_sb[0][:], qt_d[0:128, 0:512])
        for dc in range(1, 4):
            nc.sync.dma_start(ktA0_sb[dc][:], kt_d[128 * dc:128 * (dc + 1), 0:512])
            nc.sync.dma_start(qt0_sb[dc][:], qt_d[128 * dc:128 * (dc + 1), 0:512])
        nc.sync.dma_start(ktA0_sb[0][:], kt_d[0:128, 0:512])
        load_vq(0)
        nc.sync.dma_start(
            ktA1_sb[:],
            kt_d[:, 512:1024].rearrange("(dc p) k -> p dc k", p=128))
        load_vq(1)
        nc.sync.dma_start(
            qt1_sb[:],
            qt_d[:, 512:1024].rearrange("(dc p) q -> p dc q", p=128))
        nc.sync.dma_start(
            ktB_sb[:],
            kt_d[:, 1024:2048].rearrange("(dc p) k -> p dc k", p=128))
        load_vq(2)
        load_vq(3)


        fill0 = nc.gpsimd.to_reg(0.0)

        def finalize(m, o_ps, pacc_sb):
            r_ps = pst.tile([1, QB], f32, tag="st", name=f"rps{m}")
            nc.tensor.matmul(r_ps[:], onec_sb[:], pacc_sb[:],
                             start=True, stop=True)
            r_sb = pfin.tile([1, QB], f32, tag="rsb", name=f"rsb{m}")
            nc.scalar.copy(r_sb[:], r_ps[:])
            for j in range(2):
                rt_ps = pst.tile([128, 1], f32, tag="st", name=f"rt{m}_{j}")
                nc.tensor.transpose(rt_ps[:], r_sb[0:1, 128 * j:128 * (j + 1)],
                                    ident[:])
                rinv = pfin.tile([128, 1], f32, tag="rinv", name=f"rinv{m}_{j}")
                nc.vector.reciprocal(rinv[:], rt_ps[:])
                o_sb = pfin.tile([128, D], f32, tag="osb", name=f"osb{m}_{j}")
                nc.vector.tensor_scalar_mul(o_sb[:], o_ps[j][:], rinv[:])
                nc.sync.dma_start(
                    out_d[QB * m + 128 * j:QB * m + 128 * (j + 1), :], o_sb[:])

        for phase, (m0, m1) in enumerate(((0, 1), (2, 3))):
            o_ps = {m: [pacc.tile([128, D], f32, tag=f"o{m % 2}_{j}",
                                  name=f"o{m}_{j}") for j in range(2)]
                    for m in (m0, m1)}
            pacc_sb = {m: pfin.tile([128, QB], accdt, tag=f"pacc{m % 2}",
                                    name=f"pacc{m}") for m in (m0, m1)}
            e0, e1 = EXT[m0], EXT[m1]
            for c in range(e1):
                paired = c < e0
                width = 2 * QB if paired else QB
                mb = m0 if paired else m1
                st = pst.tile([128, width], f32, tag="st", name=f"st{phase}_{c}")
                for dc in range(4):
                    nc.tensor.matmul(st[:], kt_slice(dc, c),
                                     qt_slice(dc, mb, width),
                                     start=(dc == 0), stop=(dc == 3))
                pt = ppt.tile([128, width], f32r, tag="pt", name=f"pt{phase}_{c}")
                nc.scalar.activation(pt[:], st[:], Exp, scale=SCALE,
                                     bias=bias_sb[:, c:c + 1])
                # causal band masking per block present in this tile
                for bi, m in enumerate((m0, m1) if paired else (mb,)):
                    if c >= EXT[m] - BAND:
                        off = 0 if m == mb else QB * (m - mb)
                        nc.gpsimd.affine_select(
                            pt[:, off:off + QB], pt[:, off:off + QB],
                            pattern=[[2, QB]],
                            compare_op=mybir.AluOpType.is_ge, fill=fill0,
                            base=512 * m - 128 * c + 1, channel_multiplier=-1)
                # MM2: O accumulation for each block covered by this tile
                for m in ((m0, m1) if paired else (mb,)):
                    off = QB * (m - mb)
                    for j in range(2):
                        nc.tensor.matmul(
                            o_ps[m][j][:],
                            pt[:, off + 128 * j:off + 128 * (j + 1)],
                            vq_sb[c // 4][:, c % 4, :],
                            start=(c == 0), stop=(c == EXT[m] - 1))
                    if c == 0:
                        nc.vector.tensor_copy(pacc_sb[m][:],
                                              pt[:, off:off + QB])
                    else:
                        nc.vector.tensor_add(pacc_sb[m][:], pacc_sb[m][:],
                                             pt[:, off:off + QB])
                if c == e0 - 1:
                    finalize(m0, o_ps[m0], pacc_sb[m0])
            finalize(m1, o_ps[m1], pacc_sb[m1])
    nc.compile()
    return nc


def _prep_core_inputs(Q, K, V, key_mask, b, p, npdt=np.float32):
    s = 1 - p
    qt = np.ascontiguousarray(Q[p::2, b, :].T)            # [D, QL]
    kshift = np.zeros((SK, D), dtype=np.float32)
    vshift = np.zeros((SK, D), dtype=np.float32)
    kshift[s:] = K[:SK - s, b, :]
    vshift[s:] = V[:SK - s, b, :]
    valid = np.zeros(SK, dtype=bool)
    valid[s:] = ~key_mask[:SK - s, b]
    vshift[~valid] = 0.0
    bias2d = np.where(valid, 0.0, -1e30).astype(np.float32)
    bias2d = bias2d.reshape(NKC, 128).T                    # [128, NKC]
    return {
        "qt": np.ascontiguousarray(qt.astype(npdt)),
        "kt": np.ascontiguousarray(kshift.T.astype(npdt)),  # [D, SK]
        "v": vshift.astype(npdt),
        "bias2d": np.ascontiguousarray(bias2d),
        "onecol": np.ones((128, 1), dtype=np.float32),  # float32r == f32 bytes
    }


MMDT = "float16"


_orig_sprun = subprocess.run


def _ldwopt_sprun(cmd, *a, **k):
    if isinstance(cmd, list):
        cmd = ["--enable-ldw-opt=true" if c == "--enable-ldw-opt=false" else c
               for c in cmd]
    return _orig_sprun(cmd, *a, **k)


def run(inputs, trace=False, trace_cores=None):
    if os.environ.get("LDWOPT") == "1":
        subprocess.run = _ldwopt_sprun
    if "nc" not in _cache:
        _cache["nc"] = _build(mmdt=MMDT)
    nc = _cache["nc"]
    npdt = np.float16 if MMDT == "float16" else np.float32

    Q = np.asarray(inputs["Q"], dtype=np.float32)
    K = np.asarray(inputs["K"], dtype=np.float32)
    V = np.asarray(inputs["V"], dtype=np.float32)
    key_mask = np.asarray(inputs["key_mask"], dtype=bool)

    in_maps = []
    for core in range(N_CORES):
        b, p = divmod(core, 2)
        in_maps.append(_prep_core_inputs(Q, K, V, key_mask, b, p, npdt))

    try:
        res = run_bass_kernel_spmd(nc, in_maps, list(range(N_CORES)),
                                   trace=trace, trace_cores=trace_cores)
    except Exception:
        res = run_bass_kernel_spmd(nc, in_maps, list(range(N_CORES)),
                                   trace=trace, trace_cores=trace_cores)

    out = np.empty((SQ, B, D), dtype=np.float32)
    for core in range(N_CORES):
        b, p = divmod(core, 2)
        out[p::2, b, :] = res.results[core]["out"]
    return out, res


def kernel(**inputs):
    out, _ = run(inputs, trace=False)
    return out



# revision 3
# speedup vs baseline: 1.3536x; 1.3536x over previous
"""Causal single-head attention [Sq,B,D]=[2048,4,512] fp32 on 8 TRN2 NeuronCores.

Sharding: core = 2*b + p  (b = batch 0..3, p = query-row parity).
Core (b, p) computes output rows i = 2j + p (j = 0..1023) of batch b.

Key trick for SPMD (one program, 8 cores): queries are strided by 2, and
K/V are host-shifted by s = 1-p rows. Then the causal condition
  k <= i  ==  k' <= 2*j + 1   (k' = shifted key index)
is identical on every core, so the on-device causal mask is a compile-time
affine_select and block extents are core-invariant.

Math per core: S^T[k',j] = K'^T Q^T / sqrt(D) via PE (contract d);
P^T = exp(S^T) (no max subtraction: scores ~ N(0,1), bounded);
causal zeroing via affine_select; O = P V' and r = P @ ones' accumulated
over k' chunks; O /= r. Key-mask + shift padding are folded into V' rows
and the exp bias (-1e30) on the host, so masked keys contribute 0 to both
numerator and denominator.

v2 scheduling: input DMAs are issued from all five engine queues in
consumption order (parallel descriptor issue); the main loop software-
pipelines MM1 two chunk-iterations ahead of MM2 so the PE never waits on
the scalar exp / gpsimd mask chain; r is accumulated in fp16 on DVE; the
finalize chain is split across PE/ACT/DVE with its PE ops deferred one
iteration, and each block's output leaves as a single batched DMA.
"""
import math
import os
import subprocess
from contextlib import ExitStack

import numpy as np

import concourse.bass as bass
import concourse.tile as tile
import concourse.mybir as mybir
from concourse import bacc
from concourse.bass_utils import run_bass_kernel_spmd

SQ, SK, B, D = 2048, 2048, 4, 512
N_CORES = 8
QL = SQ // 2          # local q rows per core
QB = 256              # local q-block size
NBLK = QL // QB       # 4 blocks
NKC = SK // 128       # 16 key chunks
EXT = [4 * (m + 1) for m in range(NBLK)]   # k'-chunk extent per block
BAND = 4              # diagonal band width in chunks
SCALE = 1.0 / math.sqrt(D)

_cache = {}


def _build(num_devices=N_CORES, mmdt="float16"):
    f32 = mybir.dt.float32
    f16 = {"float16": mybir.dt.float16,
           "float32r": mybir.dt.float32r}[mmdt]   # matmul operand dtype
    Exp = mybir.ActivationFunctionType.Exp
    Copy = mybir.ActivationFunctionType.Copy

    nc = bacc.Bacc("TRN2", target_bir_lowering=False, debug=False,
                   num_devices=num_devices)
    qt_d = nc.dram_tensor("qt", [D, QL], f16, kind="ExternalInput").ap()
    kt_d = nc.dram_tensor("kt", [D, SK], f16, kind="ExternalInput").ap()
    v_d = nc.dram_tensor("v", [SK, D], f16, kind="ExternalInput").ap()
    bias_d = nc.dram_tensor("bias2d", [128, NKC], f32, kind="ExternalInput").ap()
    onec_d = nc.dram_tensor("onecol", [128, 1], f16, kind="ExternalInput").ap()
    out_d = nc.dram_tensor("out", [QL, D], f32, kind="ExternalOutput").ap()

    with tile.TileContext(nc) as tc, ExitStack() as ctx:
        const = ctx.enter_context(tc.tile_pool(name="const", bufs=1))
        pin = ctx.enter_context(tc.tile_pool(name="pin", bufs=1))
        ppt = ctx.enter_context(tc.tile_pool(name="ppt", bufs=3))
        pst = ctx.enter_context(tc.tile_pool(name="pst", bufs=4, space="PSUM"))
        pacc = ctx.enter_context(tc.tile_pool(name="pacc", bufs=1, space="PSUM"))
        pfin = ctx.enter_context(tc.tile_pool(name="pfin", bufs=2))

        ident = const.tile([1, 1], f32)
        bias_sb = const.tile([128, NKC], f32)
        onec_sb = const.tile([128, 1], f16)

        ktA0_sb = pin.tile([128, 4, 512], f16, tag="ktA0", name="ktA0")
        ktA1_sb = pin.tile([128, 4, 512], f16, tag="ktA1", name="ktA1")
        ktB_sb = pin.tile([128, 4, 1024], f16, tag="ktB", name="ktB")
        qt0_sb = pin.tile([128, 4, 512], f16, tag="qt0", name="qt0")
        qt1_sb = pin.tile([128, 4, 512], f16, tag="qt1", name="qt1")
        vq_sb = [pin.tile([128, 4, 512], f16, tag=f"vq{g}", name=f"vq{g}")
                 for g in range(4)]

        def kt_slice(dc, c):
            if c < 4:
                return ktA0_sb[:, dc, 128 * c:128 * (c + 1)]
            if c < 8:
                return ktA1_sb[:, dc, 128 * (c - 4):128 * (c - 3)]
            return ktB_sb[:, dc, 128 * (c - 8):128 * (c - 7)]

        def qt_slice(dc, m, width=QB):
            if m < 2:
                return qt0_sb[:, dc, QB * m:QB * m + width]
            return qt1_sb[:, dc, QB * (m - 2):QB * (m - 2) + width]

        # Parallel DMA issue across the three DMA-capable queues (SP, ACT,
        # PL), in consumption order, loads ahead of any compute per queue.
        nc.sync.dma_start(
            ktA0_sb[:],
            kt_d[:, 0:512].rearrange("(dc p) k -> p dc k", p=128))
        nc.scalar.dma_start(
            qt0_sb[:],
            qt_d[:, 0:512].rearrange("(dc p) q -> p dc q", p=128))
        nc.gpsimd.dma_start(
            vq_sb[0][:],
            v_d[0:512, :].rearrange("(c p) d -> p c d", p=128))
        nc.gpsimd.dma_start(bias_sb[:], bias_d[:])
        nc.scalar.dma_start(
            ktA1_sb[:],
            kt_d[:, 512:1024].rearrange("(dc p) k -> p dc k", p=128))
        nc.gpsimd.dma_start(
            vq_sb[1][:],
            v_d[512:1024, :].rearrange("(c p) d -> p c d", p=128))
        nc.scalar.dma_start(onec_sb[:], onec_d[:])
        nc.sync.dma_start(
            qt1_sb[:],
            qt_d[:, 512:1024].rearrange("(dc p) q -> p dc q", p=128))
        nc.sync.dma_start(
            ktB_sb[:],
            kt_d[:, 1024:2048].rearrange("(dc p) k -> p dc k", p=128))
        nc.sync.dma_start(
            vq_sb[2][:],
            v_d[1024:1536, :].rearrange("(c p) d -> p c d", p=128))
        nc.sync.dma_start(
            vq_sb[3][:],
            v_d[1536:2048, :].rearrange("(c p) d -> p c d", p=128))

        nc.vector.memset(ident[:], 1.0)
        fill0 = nc.gpsimd.to_reg(0.0)

        # Iteration list: one entry per (phase, c); paired while c < e0.
        iters = []
        for phase, (m0, m1) in enumerate(((0, 1), (2, 3))):
            for c in range(EXT[m1]):
                iters.append((phase, m0, m1, c))

        o_ps = {}
        pacc_sb = {}
        st_t = {}
        pt_t = {}
        fin_state = {}

        def mm1(i):
            phase, m0, m1, c = iters[i]
            paired = c < EXT[m0]
            width = 2 * QB if paired else QB
            mb = m0 if paired else m1
            st = pst.tile([128, width], f32, tag="st", name=f"st{phase}_{c}")
            st_t[i] = st
            for dc in range(4):
                nc.tensor.matmul(st[:], kt_slice(dc, c),
                                 qt_slice(dc, mb, width),
                                 start=(dc == 0), stop=(dc == 3))

        def exp_mask(i):
            phase, m0, m1, c = iters[i]
            paired = c < EXT[m0]
            width = 2 * QB if paired else QB
            mb = m0 if paired else m1
            st = st_t.pop(i)
            pt = ppt.tile([128, width], f16, tag="pt", name=f"pt{phase}_{c}")
            pt_t[i] = pt
            nc.scalar.activation(pt[:], st[:], Exp, scale=SCALE,
                                 bias=bias_sb[:, c:c + 1])
            for m in ((m0, m1) if paired else (mb,)):
                if c >= EXT[m] - BAND:
                    off = QB * (m - mb)
                    nc.gpsimd.affine_select(
                        pt[:, off:off + QB], pt[:, off:off + QB],
                        pattern=[[2, QB]],
                        compare_op=mybir.AluOpType.is_ge, fill=fill0,
                        base=512 * m - 128 * c + 1, channel_multiplier=-1)

        def mm2(i):
            phase, m0, m1, c = iters[i]
            paired = c < EXT[m0]
            mb = m0 if paired else m1
            pt = pt_t.pop(i)
            if c == 0:
                for m in (m0, m1):
                    o_ps[m] = [pacc.tile([128, D], f32, tag=f"o{m % 2}_{j}",
                                         name=f"o{m}_{j}") for j in range(2)]
                    pacc_sb[m] = pfin.tile([128, QB], f16, tag=f"pacc{m % 2}",
                                           name=f"pacc{m}")
            for m in ((m0, m1) if paired else (mb,)):
                off = QB * (m - mb)
                for j in range(2):
                    nc.tensor.matmul(
                        o_ps[m][j][:],
                        pt[:, off + 128 * j:off + 128 * (j + 1)],
                        vq_sb[c // 4][:, c % 4, :],
                        start=(c == 0), stop=(c == EXT[m] - 1))
                if c == 0:
                    nc.vector.tensor_copy(pacc_sb[m][:], pt[:, off:off + QB])
                else:
                    nc.vector.tensor_add(pacc_sb[m][:], pacc_sb[m][:],
                                         pt[:, off:off + QB])

        def fin_a(m):
            # r = sum_k P (PE, contract partitions via ones) + PSUM->SBUF copy
            r_ps = pst.tile([1, QB], f32, tag="st", name=f"rps{m}")
            nc.tensor.matmul(r_ps[:], onec_sb[:], pacc_sb[m][:],
                             start=True, stop=True)
            r_sb = pfin.tile([1, QB], f32, tag="rsb", name=f"rsb{m}")
            nc.scalar.copy(r_sb[:], r_ps[:])
            fin_state[m] = r_sb

        def fin_b(m):
            r_sb = fin_state.pop(m)
            o_sb = pfin.tile([128, 2, D], f32, tag="osb", name=f"osb{m}")
            rinv = []
            for j in range(2):
                rt_ps = pst.tile([128, 1], f32, tag="st", name=f"rt{m}_{j}")
                nc.tensor.transpose(rt_ps[:], r_sb[0:1, 128 * j:128 * (j + 1)],
                                    ident[:])
                ri = pfin.tile([128, 1], f32, tag="rinv", name=f"rinv{m}_{j}")
                nc.vector.reciprocal(ri[:], rt_ps[:])
                rinv.append(ri)
            nc.vector.tensor_scalar_mul(o_sb[:, 0, :], o_ps[m][0][:], rinv[0][:])
            nc.scalar.activation(o_sb[:, 1, :], o_ps[m][1][:], Copy,
                                 scale=rinv[1][:])
            nc.sync.dma_start(
                out_d[QB * m:QB * (m + 1), :].rearrange("(j p) d -> p j d",
                                                        p=128),
                o_sb[:])

        pending_b = []
        n = len(iters)
        mm1(0)
        if n > 1:
            mm1(1)
        for i in range(n):
            phase, m0, m1, c = iters[i]
            if i + 2 < n:
                mm1(i + 2)
            while pending_b:
                fin_b(pending_b.pop(0))
            exp_mask(i)
            mm2(i)
            if c == EXT[m0] - 1:
                fin_a(m0)
                pending_b.append(m0)
            if c == EXT[m1] - 1:
                fin_a(m1)
                pending_b.append(m1)
        while pending_b:
            fin_b(pending_b.pop(0))
    nc.compile()
    return nc


def _prep_core_inputs(Q, K, V, key_mask, b, p, npdt=np.float16):
    s = 1 - p
    qt = np.ascontiguousarray(Q[p::2, b, :].T)            # [D, QL]
    kshift = np.zeros((SK, D), dtype=np.float32)
    vshift = np.zeros((SK, D), dtype=np.float32)
    kshift[s:] = K[:SK - s, b, :]
    vshift[s:] = V[:SK - s, b, :]
    valid = np.zeros(SK, dtype=bool)
    valid[s:] = ~key_mask[:SK - s, b]
    vshift[~valid] = 0.0
    bias2d = np.where(valid, 0.0, -1e30).astype(np.float32)
    bias2d = bias2d.reshape(NKC, 128).T                    # [128, NKC]
    return {
        "qt": np.ascontiguousarray(qt.astype(npdt)),
        "kt": np.ascontiguousarray(kshift.T.astype(npdt)),  # [D, SK]
        "v": vshift.astype(npdt),
        "bias2d": np.ascontiguousarray(bias2d),
        "onecol": np.ones((128, 1), dtype=npdt),
    }


MMDT = "float16"


_orig_sprun = subprocess.run


def _ldwopt_sprun(cmd, *a, **k):
    if isinstance(cmd, list):
        cmd = ["--enable-ldw-opt=true" if c == "--enable-ldw-opt=false" else c
               for c in cmd]
    return _orig_sprun(cmd, *a, **k)


def run(inputs, trace=False, trace_cores=None):
    if os.environ.get("LDWOPT") == "1":
        subprocess.run = _ldwopt_sprun
    if "nc" not in _cache:
        _cache["nc"] = _build(mmdt=MMDT)
    nc = _cache["nc"]
    npdt = np.float16 if MMDT == "float16" else np.float32

    Q = np.asarray(inputs["Q"], dtype=np.float32)
    K = np.asarray(inputs["K"], dtype=np.float32)
    V = np.asarray(inputs["V"], dtype=np.float32)
    key_mask = np.asarray(inputs["key_mask"], dtype=bool)

    in_maps = []
    for core in range(N_CORES):
        b, p = divmod(core, 2)
        in_maps.append(_prep_core_inputs(Q, K, V, key_mask, b, p, npdt))

    try:
        res = run_bass_kernel_spmd(nc, in_maps, list(range(N_CORES)),
                                   trace=trace, trace_cores=trace_cores)
    except Exception:
        res = run_bass_kernel_spmd(nc, in_maps, list(range(N_CORES)),
                                   trace=trace, trace_cores=trace_cores)

    out = np.empty((SQ, B, D), dtype=np.float32)
    for core in range(N_CORES):
        b, p = divmod(core, 2)
        out[p::2, b, :] = res.results[core]["out"]
    return out, res


def kernel(**inputs):
    out, _ = run(inputs, trace=False)
    return out
